# revision 11
# baseline (speedup 1.0000x reference)
"""RPE multi-head attention Trainium2 kernel (8 NeuronCores, SPMD).

Reference computation (b=2, n=m=512, d=256, h=8, c=32):
    q = (input_q @ Wq + bq)   -> [b, h, n, c]
    k = (input_k @ Wk + bk)   -> [b, h, m, c]
    v = (input_v @ Wv + bv)   -> [b, h, m, c]
    p = (embed_qk @ Wp + bp)  -> [b, n, m, h, c]
    scores = (q.p + q.k) / sqrt(c); attn = softmax_m(scores); hidden = attn @ v

Key algebraic rewrite: scores_p[b,h,n,m] = sum_d E[b,n,m,d] * t[b,h,n,d]
with t[b,h,n,d] = sum_c Wp[d, h*32+c] * q[b,h,n,c].  Contracting Wp with q
first drops the dominant flops from 68.7 GF (embed @ Wp) to ~2.1 GF and the
kernel becomes HBM-bound on the single read of embed_qk (512 MiB).
The bp bias term is constant over m, so it cancels in softmax exactly and is
dropped (it affects neither attn nor hidden).

Sharding: split n across the 8 cores (64 rows each, both batches); k/v work is
tiny and replicated.  No collectives.
"""

import math
import sys

import numpy as np

if "/opt/trn_rl_repo" not in sys.path:
    sys.path.insert(0, "/opt/trn_rl_repo")

import concourse.bass as bass
import concourse.mybir as mybir
import concourse.tile as tile
from concourse import bacc
from concourse.bass_utils import run_bass_kernel_spmd
from concourse.masks import make_identity

F32 = mybir.dt.float32
B, N, M, D, H, C = 2, 512, 512, 256, 8, 32
NCORES = 8
NL = N // NCORES          # 64 n-rows per core (per batch)
NPAIR = B * NL            # 128 (b, n) pairs per core
SCALE = 1.0 / math.sqrt(C)

_CACHED_NC = None


def build_nc():
    nc = bacc.Bacc("TRN2", debug=False, num_devices=NCORES)
    Exp = mybir.ActivationFunctionType.Exp
    X = mybir.AxisListType.X

    emb_p = nc.declare_dram_parameter("emb", [NPAIR, 128, 2, M], F32, isOutput=False)
    qiT_p = nc.declare_dram_parameter("qiT", [2, 128, NPAIR], F32, isOutput=False)
    kiT_p = nc.declare_dram_parameter("kiT", [B, 2, 128, M], F32, isOutput=False)
    viT_p = nc.declare_dram_parameter("viT", [B, 2, 128, M], F32, isOutput=False)
    wq_p = nc.declare_dram_parameter("wq", [2, 128, D], F32, isOutput=False)
    wk_p = nc.declare_dram_parameter("wk", [2, 128, D], F32, isOutput=False)
    wv_p = nc.declare_dram_parameter("wv", [2, 128, D], F32, isOutput=False)
    wpT_p = nc.declare_dram_parameter("wpT", [2, 128, D], F32, isOutput=False)
    bq_p = nc.declare_dram_parameter("bq", [D], F32, isOutput=False)
    bk_p = nc.declare_dram_parameter("bk", [D], F32, isOutput=False)
    bv_p = nc.declare_dram_parameter("bv", [D], F32, isOutput=False)
    attn_o = nc.declare_dram_parameter("attn_o", [B, H, NL, M], F32, isOutput=True)
    hid_o = nc.declare_dram_parameter("hid_o", [B, NL, D], F32, isOutput=True)

    def mm(out, lhsT, rhs, start, stop):
        nc.tensor.matmul(out, lhsT=lhsT, rhs=rhs, start=start, stop=stop)

    def tp(out, in_, ident):
        nc.tensor.transpose(out, in_, ident)

    with tile.TileContext(nc) as tc:
        with (
            tc.tile_pool(name="const", bufs=1) as const,
            tc.tile_pool(name="persist", bufs=1) as persist,
            tc.tile_pool(name="stage", bufs=2) as stage,
            tc.tile_pool(name="embp", bufs=12) as embp,
            tc.tile_pool(name="spsb", bufs=3) as spsb,
            tc.tile_pool(name="spin", bufs=2) as spin,
            tc.tile_pool(name="work", bufs=2) as work,
            tc.tile_pool(name="small", bufs=3) as small,
            tc.tile_pool(name="atp", bufs=2) as atp,
            tc.tile_pool(name="hidp", bufs=2) as hidp,
            tc.tile_pool(name="ps1", bufs=1, space="PSUM") as ps1,
            tc.tile_pool(name="ps_sp", bufs=2, space="PSUM") as ps_sp,
            tc.tile_pool(name="ps_se", bufs=2, space="PSUM") as ps_se,
            tc.tile_pool(name="ps_tr", bufs=2, space="PSUM") as ps_tr,
            tc.tile_pool(name="dram", bufs=1, space="DRAM") as dram,
        ):
            # ---- constants ----
            def load2(pool, p, tag, shape):
                ts_ = []
                for c_ in range(2):
                    t = pool.tile(shape, F32, tag=f"{tag}{c_}", name=f"{tag}{c_}")
                    nc.sync.dma_start(out=t, in_=p[c_])
                    ts_.append(t)
                return ts_

            wq_sb = load2(const, wq_p, "wq", [128, D])
            wk_sb = load2(const, wk_p, "wk", [128, D])
            wv_sb = load2(const, wv_p, "wv", [128, D])
            wpT_sb = []
            for j in range(4):
                t = const.tile([64, D], F32, tag=f"wpT{j}", name=f"wpT{j}")
                nc.sync.dma_start(out=t, in_=wpT_p[j // 2][(j % 2) * 64:(j % 2) * 64 + 64, :])
                wpT_sb.append(t)

            def loadb(p, tag):
                ts_ = []
                for c_ in range(2):
                    t = const.tile([128, 1], F32, tag=f"{tag}{c_}", name=f"{tag}{c_}")
                    nc.sync.dma_start(
                        out=t, in_=p[c_ * 128:(c_ + 1) * 128].rearrange("(p o) -> p o", o=1)
                    )
                    ts_.append(t)
                return ts_

            bq_sb = loadb(bq_p, "bq")
            bk_sb = loadb(bk_p, "bk")
            bvb = const.tile([128, D], F32, tag="bvb")
            nc.gpsimd.dma_start(
                out=bvb,
                in_=bass.AP(tensor=bv_p[:].tensor, offset=0, ap=[[0, 128], [1, D]]),
            )
            ident = const.tile([64, 64], F32, tag="ident")
            make_identity(nc, ident)

            qiT_sb = []
            for c_ in range(2):
                t = const.tile([128, NPAIR], F32, tag=f"qiT{c_}", name=f"qiT{c_}")
                nc.sync.dma_start(out=t, in_=qiT_p[c_])
                qiT_sb.append(t)

            # ---- qT[d_out, bn] = (input_q @ Wq + bq)^T : four [64, NPAIR] tiles ----
            qT_sb = [
                persist.tile([64, NPAIR], F32, tag=f"qT{j}", name=f"qT{j}")
                for j in range(4)
            ]
            for oc in range(2):
                ps = ps1.tile([128, NPAIR], F32, tag="pp")
                for ic in range(2):
                    mm(ps, wq_sb[ic][:, oc * 128:(oc + 1) * 128], qiT_sb[ic],
                       ic == 0, ic == 1)
                for half in range(2):
                    nc.vector.tensor_scalar_add(
                        qT_sb[oc * 2 + half], ps[half * 64:(half + 1) * 64, :],
                        bq_sb[oc][half * 64:(half + 1) * 64, :],
                    )

            # ---- kT[b][d_out, m], v[b][m, d_out] ----
            kT_sb = [persist.tile([64, B, M], F32, tag=f"kT{j}", name=f"kT{j}") for j in range(4)]
            v_sb = persist.tile([128, B, 4, D], F32, tag="v")
            for b in range(B):
                kiT_st = []
                for ic in range(2):
                    t = stage.tile([128, M], F32, tag="stg", name="stg")
                    nc.sync.dma_start(out=t, in_=kiT_p[b, ic])
                    kiT_st.append(t)
                for oc in range(2):
                    ps = ps1.tile([128, M], F32, tag="pp")
                    for ic in range(2):
                        mm(ps, wk_sb[ic][:, oc * 128:(oc + 1) * 128], kiT_st[ic],
                           ic == 0, ic == 1)
                    for half in range(2):
                        nc.vector.tensor_scalar_add(
                            kT_sb[oc * 2 + half][:, b, :],
                            ps[half * 64:(half + 1) * 64, :],
                            bk_sb[oc][half * 64:(half + 1) * 64, :],
                        )
                viT_st = []
                for ic in range(2):
                    t = stage.tile([128, M], F32, tag="stg", name="stg")
                    nc.sync.dma_start(out=t, in_=viT_p[b, ic])
                    viT_st.append(t)
                for mc in range(4):
                    ps = ps1.tile([128, D], F32, tag="pp")
                    for ic in range(2):
                        mm(ps, viT_st[ic][:, mc * 128:(mc + 1) * 128], wv_sb[ic],
                           ic == 0, ic == 1)
                    nc.vector.tensor_add(v_sb[:, b, mc, :], ps, bvb)

            # ---- Tmat[dp, dc, bn, h] = sum_c Wp[d, h*32+c] * q[bn, h*32+c] ----
            Tmat = persist.tile([128, 2, NPAIR, H], F32, tag="Tmat")
            for h in range(H):
                j, off = h // 2, (h % 2) * 32
                for dc in range(2):
                    ps = ps1.tile([128, NPAIR], F32, tag="pp")
                    mm(ps, wpT_sb[j][off:off + 32, dc * 128:(dc + 1) * 128],
                       qT_sb[j][off:off + 32, :], True, True)
                    nc.vector.tensor_copy(Tmat[:, dc, :, h], ps)

            # ---- main: scores_p per (b, n) -> DRAM; then per (b, h) softmax ----
            for b in range(B):
                spd = dram.tile([H, NL, M], F32, tag=f"spd{b}")
                # phase F: scores_p
                spp = None
                for p64 in range(NL):
                    pair = b * NL + p64
                    if p64 % 2 == 0:
                        spp = ps_sp.tile([40, M], F32, tag="sp")
                    et = embp.tile([128, 2, M], F32, tag="emb")
                    nc.sync.dma_start(out=et, in_=emb_p[pair])
                    r = (p64 % 2) * 32
                    for dc in range(2):
                        mm(spp[r:r + 8, :], Tmat[:, dc, pair, :], et[:, dc, :],
                           dc == 0, dc == 1)
                    if p64 % 2 == 1:
                        sps = spsb.tile([40, M], F32, tag="sps")
                        for q_ in range(2):
                            nc.vector.tensor_copy(
                                sps[q_ * 32:q_ * 32 + 8, :], spp[q_ * 32:q_ * 32 + 8, :]
                            )
                            nc.sync.dma_start(
                                out=spd[:, p64 - 1 + q_, :],
                                in_=sps[q_ * 32:q_ * 32 + 8, :],
                            )
                # phase G: scores_e + softmax + attn@v
                hid_sb = hidp.tile([NL, D], F32, tag="hid")
                for h in range(H):
                    j, off = h // 2, (h % 2) * 32
                    se = ps_se.tile([NL, M], F32, tag="se")
                    mm(se, qT_sb[j][off:off + 32, b * NL:(b + 1) * NL],
                       kT_sb[j][off:off + 32, b, :], True, True)
                    spi = spin.tile([NL, M], F32, tag="spi")
                    nc.sync.dma_start(out=spi, in_=spd[h])
                    ssb = work.tile([NL, M], F32, tag="ssb")
                    nc.vector.tensor_add(ssb, se, spi)
                    mx = small.tile([NL, 1], F32, tag="mx")
                    nc.vector.reduce_max(mx, ssb, axis=X)
                    nmx = small.tile([NL, 1], F32, tag="nmx")
                    nc.vector.tensor_scalar_mul(nmx, mx, -SCALE)
                    pex = work.tile([NL, M], F32, tag="pex")
                    ssum = small.tile([NL, 1], F32, tag="ssum")
                    nc.scalar.activation(pex, ssb, Exp, bias=nmx, scale=SCALE, accum_out=ssum)
                    rin = small.tile([NL, 1], F32, tag="rin")
                    nc.vector.reciprocal(rin, ssum)
                    att = work.tile([NL, M], F32, tag="att")
                    nc.vector.tensor_scalar_mul(att, pex, rin)
                    nc.sync.dma_start(out=attn_o[b, h], in_=att)
                    trp = ps_tr.tile([128, 4, NL], F32, tag="tr")
                    for mc in range(4):
                        tp(trp[:, mc, :], att[:, mc * 128:(mc + 1) * 128], ident)
                    atT = atp.tile([128, 4, NL], F32, tag="atT")
                    nc.vector.tensor_copy(atT, trp)
                    hpp = ps_se.tile([NL, C], F32, tag="se")
                    for mc in range(4):
                        mm(hpp, atT[:, mc, :], v_sb[:, b, mc, h * C:(h + 1) * C],
                           mc == 0, mc == 3)
                    nc.vector.tensor_copy(hid_sb[:, h * C:(h + 1) * C], hpp)
                nc.sync.dma_start(out=hid_o[b], in_=hid_sb)
    nc.finalize()
    return nc


def _prep_core_inputs(inputs):
    """Host-side shard + layout prep. Returns list of 8 input maps."""
    iq = np.ascontiguousarray(np.asarray(inputs["input_q"], dtype=np.float32))
    ik = np.ascontiguousarray(np.asarray(inputs["input_k"], dtype=np.float32))
    iv = np.ascontiguousarray(np.asarray(inputs["input_v"], dtype=np.float32))
    emb = np.asarray(inputs["embed_qk"], dtype=np.float32)
    Wq = np.asarray(inputs["Wq"], dtype=np.float32)
    Wk = np.asarray(inputs["Wk"], dtype=np.float32)
    Wv = np.asarray(inputs["Wv"], dtype=np.float32)
    Wp = np.asarray(inputs["Wp"], dtype=np.float32)
    bq = np.asarray(inputs["bq"], dtype=np.float32)
    bk = np.asarray(inputs["bk"], dtype=np.float32)
    bv = np.asarray(inputs["bv"], dtype=np.float32)

    wq_h = np.ascontiguousarray(Wq.reshape(2, 128, D))
    wk_h = np.ascontiguousarray(Wk.reshape(2, 128, D))
    wv_h = np.ascontiguousarray(Wv.reshape(2, 128, D))
    wpT_h = np.ascontiguousarray(Wp.T.reshape(2, 128, D))
    # kiT[b, dc, dp, m] = input_k[b, m, dc*128+dp]
    kiT = np.ascontiguousarray(ik.transpose(0, 2, 1).reshape(B, 2, 128, M))
    viT = np.ascontiguousarray(iv.transpose(0, 2, 1).reshape(B, 2, 128, M))

    in_maps = []
    for core in range(NCORES):
        sl = slice(core * NL, (core + 1) * NL)
        # emb[pair, dp, dc, m]; pair = b*NL + nl, d = dc*128 + dp
        e = np.ascontiguousarray(
            emb[:, sl].reshape(B, NL, M, 2, 128).transpose(0, 1, 4, 3, 2)
        ).reshape(NPAIR, 128, 2, M)
        # qiT[dc, dp, b*NL+nl] = input_q[b, n0+nl, dc*128+dp]
        qiT = np.ascontiguousarray(
            iq[:, sl].reshape(NPAIR, 2, 128).transpose(1, 2, 0)
        )
        in_maps.append({
            "emb": e, "qiT": qiT, "kiT": kiT, "viT": viT,
            "wq": wq_h, "wk": wk_h, "wv": wv_h, "wpT": wpT_h,
            "bq": bq, "bk": bk, "bv": bv,
        })
    return in_maps


def kernel(**inputs):
    global _CACHED_NC
    if _CACHED_NC is None:
        _CACHED_NC = build_nc()
    nc = _CACHED_NC
    in_maps = _prep_core_inputs(inputs)
    res = run_bass_kernel_spmd(nc, in_maps, list(range(NCORES)))
    attn = np.empty((B, H, N, M), np.float32)
    hidden = np.empty((B, N, D), np.float32)
    for core in range(NCORES):
        sl = slice(core * NL, (core + 1) * NL)
        attn[:, :, sl, :] = res.results[core]["attn_o"]
        hidden[:, sl, :] = res.results[core]["hid_o"]
    return hidden, attn
